# revision 3
# baseline (speedup 1.0000x reference)
"""Trainium2 Bass kernel for nn_MergeHeads (moe_routing).

Computes out[t] = sum_a p[t,a] * (x[t,a] @ W[idx[t,a]] + b[idx[t,a]])
for B*S = 16384 tokens, A=2 slots, H=8 heads, DH=128, DM=2048.

Strategy: data-parallel over tokens across 8 NeuronCores (2048 tokens
each); W/b replicated.  Per core, dense routed formulation:
  routedT[h][d, t] = sum_a (idx[t,a]==h) * p[t,a] * x[t,a,d]
built with DVE mask-multiplies (token-on-partition layout, per-partition
scalars) + one PE transpose per (tile, head) accumulating into PSUM;
the mandatory PSUM->SBUF copy rounds to float32r.  Then per 512-wide
output chunk: 8 accumulated float32r matmuls (K=128, tokens stationary,
W moving N=512) plus one K=8 bias matmul (lhsT = transposed per-head
prob sums, rhs = b).  float32r runs at full PE rate (1 cyc/row at
N>=256) with ~1.6e-4 max relative error vs fp32.
"""

import os
import numpy as np

B, S, A, H, DH, DM = 4, 4096, 2, 8, 128, 2048
NCORES = 8
T = B * S
TLOC = T // NCORES        # 2048 tokens per core
P = 128                   # partitions / token tile
NT = TLOC // P            # 16 token tiles per core
NFREE = 512               # matmul moving free dim (one PSUM bank of fp32)
MC = DM // NFREE          # 4 output chunks per token tile

# compute dtype: "f32r" (default), "bf16", or "f32"
CDT_MODE = os.environ.get("TRNK_DTYPE", "f32r")

_CACHE = {}


def _build_nc():
    import concourse.mybir as mybir
    from concourse import bacc
    from concourse.tile import TileContext
    from concourse.masks import make_identity

    f32 = mybir.dt.float32
    cdt = {
        "f32r": mybir.dt.float32r,
        "bf16": mybir.dt.bfloat16,
        "f32": mybir.dt.float32,
    }[CDT_MODE]

    nc = bacc.Bacc("TRN2", target_bir_lowering=False, debug=False)

    x_d = nc.dram_tensor("x", [TLOC, A, DH], f32, kind="ExternalInput")
    idx_d = nc.dram_tensor("idxf", [TLOC, A], f32, kind="ExternalInput")
    p_d = nc.dram_tensor("p", [TLOC, A], f32, kind="ExternalInput")
    w_d = nc.dram_tensor("W", [H, DH, DM], f32, kind="ExternalInput")
    b_d = nc.dram_tensor("b", [H, DM], f32, kind="ExternalInput")
    hg_d = nc.dram_tensor("hgrid", [P, H, A], f32, kind="ExternalInput")
    y_d = nc.dram_tensor("out", [TLOC, DM], f32, kind="ExternalOutput")

    with TileContext(nc) as tc:
        with tc.tile_pool(name="const", bufs=1) as const, \
             tc.tile_pool(name="wstage", bufs=2) as wstage, \
             tc.tile_pool(name="xpool", bufs=3) as xpool, \
             tc.tile_pool(name="tmp", bufs=6) as tmppool, \
             tc.tile_pool(name="rpool", bufs=2) as rpool, \
             tc.tile_pool(name="wst", bufs=2) as wstpool, \
             tc.tile_pool(name="ypool", bufs=4) as ypool, \
             tc.tile_pool(name="pr", bufs=2, space="PSUM") as prpool, \
             tc.tile_pool(name="py", bufs=2, space="PSUM") as pypool, \
             tc.tile_pool(name="pw", bufs=2, space="PSUM") as pwpool:

            # ---- constants / setup ----
            eye = const.tile([P, P], f32, tag="eye")
            make_identity(nc, eye[:])

            hg = const.tile([P, H, A], f32, tag="hg")
            nc.sync.dma_start(hg[:], hg_d[:])

            idx_sb = const.tile([P, NT, A], f32, tag="idx")
            p_sb = const.tile([P, NT, A], f32, tag="p")
            # dst[tp, i, a] = src[i*P + tp, a]
            src_idx = idx_d[:].rearrange("(i tp) a -> tp i a", tp=P)
            src_p = p_d[:].rearrange("(i tp) a -> tp i a", tp=P)
            nc.sync.dma_start(idx_sb[:], src_idx)
            nc.sync.dma_start(p_sb[:], src_p)

            # routing weights wgt[tp, i, h, a] = (idx==h) * p
            wgt = const.tile([P, NT, H, A], f32, tag="wgt")
            idx_b = idx_sb[:].unsqueeze(2).broadcast_to([P, NT, H, A])
            p_b = p_sb[:].unsqueeze(2).broadcast_to([P, NT, H, A])
            hg_b = hg[:].unsqueeze(1).broadcast_to([P, NT, H, A])
            nc.vector.tensor_tensor(wgt[:], idx_b, hg_b, mybir.AluOpType.is_equal)
            nc.vector.tensor_tensor(wgt[:], wgt[:], p_b, mybir.AluOpType.mult)
            # per-head prob sums wsum[tp, i, h] = wgt[...,0] + wgt[...,1]
            wsum = const.tile([P, NT, H], f32, tag="wsum")
            nc.vector.tensor_tensor(
                wsum[:], wgt[:, :, :, 0], wgt[:, :, :, 1], mybir.AluOpType.add
            )

            # W -> SBUF, rounded to compute dtype: W_r[d, h, m]
            w_r = const.tile([P, H, DM], cdt, tag="w_r")
            for h in range(H):
                st = wstage.tile([P, DM], f32, tag="wst")
                nc.sync.dma_start(st[:], w_d[h, :, :])
                if h % 2 == 0:
                    nc.vector.tensor_copy(w_r[:, h, :], st[:])
                else:
                    nc.scalar.copy(w_r[:, h, :], st[:])

            # b -> SBUF rounded: b_r[h, m] (partitions 0..7)
            bstage = const.tile([H, DM], f32, tag="bstage")
            nc.sync.dma_start(bstage[:], b_d[:])
            b_r = const.tile([H, DM], cdt, tag="b_r")
            nc.vector.tensor_copy(b_r[:], bstage[:])

            # ---- main pipeline, software-pipelined by one tile ----
            prev = None  # (routedT tile, wsT tile) of tile i-1
            for i in range(NT + 1):
                cur = None
                if i < NT:
                    x_t = xpool.tile([P, A, DH], f32, tag="x")
                    nc.sync.dma_start(x_t[:], x_d[i * P:(i + 1) * P, :, :])

                    r_t = rpool.tile([P, H, DH], cdt, tag="r")
                    for g in range(2):          # head groups 0-3, 4-7
                        pr_t = prpool.tile([P, 4 * P], f32, tag="pr")
                        for hh in range(4):
                            h = g * 4 + hh
                            tmp = tmppool.tile([P, DH], f32, tag="tmp")
                            nc.vector.tensor_scalar_mul(
                                tmp[:], x_t[:, 0, :], wgt[:, i, h, 0:1]
                            )
                            nc.vector.scalar_tensor_tensor(
                                tmp[:], x_t[:, 1, :], wgt[:, i, h, 1:2], tmp[:],
                                op0=mybir.AluOpType.mult, op1=mybir.AluOpType.add,
                            )
                            nc.tensor.transpose(
                                pr_t[:, hh * P:(hh + 1) * P], tmp[:], eye[:]
                            )
                        dst = r_t[:, g * 4:(g + 1) * 4, :].rearrange("p a b -> p (a b)")
                        if g == 0:
                            nc.vector.tensor_copy(dst, pr_t[:])
                        else:
                            nc.scalar.copy(dst, pr_t[:])

                    # transposed per-head prob sums for the bias matmul
                    pw_t = pwpool.tile([H, P], f32, tag="pw")
                    nc.tensor.transpose(pw_t[:], wsum[:, i, :], eye[:])
                    ws_t = wstpool.tile([H, P], cdt, tag="ws")
                    nc.scalar.copy(ws_t[:], pw_t[:])
                    cur = (r_t, ws_t)

                if i >= 1:
                    r_p, ws_p = prev
                    j = i - 1
                    for mc in range(MC):
                        py_t = pypool.tile([P, NFREE], f32, tag="py")
                        for h in range(H):
                            nc.tensor.matmul(
                                py_t[:],
                                r_p[:, h, :],
                                w_r[:, h, mc * NFREE:(mc + 1) * NFREE],
                                start=(h == 0), stop=False,
                            )
                        nc.tensor.matmul(
                            py_t[:], ws_p[:], b_r[:, mc * NFREE:(mc + 1) * NFREE],
                            start=False, stop=True,
                        )
                        y_t = ypool.tile([P, NFREE], f32, tag="y")
                        if mc % 2 == 0:
                            nc.vector.tensor_copy(y_t[:], py_t[:])
                        else:
                            nc.scalar.copy(y_t[:], py_t[:])
                        nc.sync.dma_start(
                            y_d[j * P:(j + 1) * P, mc * NFREE:(mc + 1) * NFREE],
                            y_t[:],
                        )
                prev = cur

    nc.compile()
    return nc


def _get_nc():
    if "nc" not in _CACHE:
        _CACHE["nc"] = _build_nc()
    return _CACHE["nc"]


def kernel(embedding, sel_idx, sel_probs, W, b):
    from concourse.bass_utils import run_bass_kernel_spmd

    emb = np.ascontiguousarray(embedding, dtype=np.float32).reshape(T, A, DH)
    idxf = np.ascontiguousarray(sel_idx).reshape(T, A).astype(np.float32)
    pf = np.ascontiguousarray(sel_probs, dtype=np.float32).reshape(T, A)
    Wf = np.ascontiguousarray(W, dtype=np.float32)
    bf = np.ascontiguousarray(b, dtype=np.float32)
    hgrid = np.ascontiguousarray(
        np.broadcast_to(
            np.arange(H, dtype=np.float32)[None, :, None], (P, H, A)
        )
    )

    nc = _get_nc()
    in_maps = []
    for c in range(NCORES):
        sl = slice(c * TLOC, (c + 1) * TLOC)
        in_maps.append({
            "x": emb[sl],
            "idxf": idxf[sl],
            "p": pf[sl],
            "W": Wf,
            "b": bf,
            "hgrid": hgrid,
        })

    trace = os.environ.get("TRNK_TRACE") == "1"
    if trace:
        _register_ntff_stub()
    res = run_bass_kernel_spmd(
        nc, in_maps, core_ids=list(range(NCORES)), trace=trace
    )
    if trace:
        _CACHE["exec_time_ns"] = res.exec_time_ns
        _CACHE["results_obj"] = res

    out = np.concatenate(
        [res.results[c]["out"] for c in range(NCORES)], axis=0
    )
    return out.reshape(B, S, DM)


def _register_ntff_stub():
    """antenv.axon_hooks is absent in this image; back it with the boot
    ctypes NTFF hook so trace=True works under axon."""
    import sys, types
    try:
        import antenv.axon_hooks  # noqa: F401
        return
    except ImportError:
        pass
    try:
        import antenv
        from trn_agent_boot.trn_boot import _ntff_profile_via_ctypes
    except ImportError:
        return
    mod = types.ModuleType("antenv.axon_hooks")
    hook = [None]

    def set_axon_ntff_profile_hook(h):
        hook[0] = h

    def get_axon_ntff_profile_hook():
        if hook[0] is None:
            hook[0] = _ntff_profile_via_ctypes("/opt/axon/libaxon_pjrt.so")
        return hook[0]

    mod.set_axon_ntff_profile_hook = set_axon_ntff_profile_hook
    mod.get_axon_ntff_profile_hook = get_axon_ntff_profile_hook
    sys.modules["antenv.axon_hooks"] = mod
    antenv.axon_hooks = mod


# revision 7
# speedup vs baseline: 1.1132x; 1.1132x over previous
"""Trainium2 Bass kernel for nn_MergeHeads (moe_routing).

Computes out[t] = sum_a p[t,a] * (x[t,a] @ W[idx[t,a]] + b[idx[t,a]])
for B*S = 16384 tokens, A=2 slots, H=8 heads, DH=128, DM=2048.

Strategy: data-parallel over tokens across 8 NeuronCores (2048 tokens
each); W/b replicated.  Per core, dense routed formulation:
  routedT[h][d, t] = sum_a (idx[t,a]==h) * p[t,a] * x[t,a,d]
built with DVE mask-multiplies (token-on-partition layout, per-partition
scalars) + one PE transpose per (tile, head) accumulating into PSUM;
the mandatory PSUM->SBUF copy rounds to float32r.  Then per 512-wide
output chunk: 8 accumulated float32r matmuls (K=128, tokens stationary,
W moving N=512) plus one K=8 bias matmul (lhsT = transposed per-head
prob sums, rhs = b).  float32r runs at full PE rate (1 cyc/row at
N>=256) with ~1.6e-4 max relative error vs fp32.
"""

import os
import numpy as np

B, S, A, H, DH, DM = 4, 4096, 2, 8, 128, 2048
NCORES = 8
T = B * S
TLOC = T // NCORES        # 2048 tokens per core
P = 128                   # partitions / token tile
NT = TLOC // P            # 16 token tiles per core
NFREE = 512               # matmul moving free dim (one PSUM bank of fp32)
MC = DM // NFREE          # 4 output chunks per token tile

# compute dtype: "f32r" (default), "bf16", or "f32"
CDT_MODE = os.environ.get("TRNK_DTYPE", "f32r")

_CACHE = {}


def _build_nc():
    import concourse.mybir as mybir
    from concourse import bacc
    from concourse.tile import TileContext
    from concourse.masks import make_identity

    f32 = mybir.dt.float32
    cdt = {
        "f32r": mybir.dt.float32r,
        "bf16": mybir.dt.bfloat16,
        "f32": mybir.dt.float32,
    }[CDT_MODE]

    nc = bacc.Bacc("TRN2", target_bir_lowering=False, debug=False)

    x_d = nc.dram_tensor("x", [TLOC, A, DH], f32, kind="ExternalInput")
    idx_d = nc.dram_tensor("idxf", [TLOC, A], f32, kind="ExternalInput")
    p_d = nc.dram_tensor("p", [TLOC, A], f32, kind="ExternalInput")
    w_d = nc.dram_tensor("W", [H, DH, DM], f32, kind="ExternalInput")
    b_d = nc.dram_tensor("b", [H, DM], f32, kind="ExternalInput")
    hg_d = nc.dram_tensor("hgrid", [P, H, A], f32, kind="ExternalInput")
    y_d = nc.dram_tensor("out", [TLOC, DM], f32, kind="ExternalOutput")

    with TileContext(nc) as tc:
        with tc.tile_pool(name="const", bufs=1) as const, \
             tc.tile_pool(name="wstage", bufs=2) as wstage, \
             tc.tile_pool(name="xpool", bufs=3) as xpool, \
             tc.tile_pool(name="tmp", bufs=6) as tmppool, \
             tc.tile_pool(name="rpool", bufs=2) as rpool, \
             tc.tile_pool(name="wst", bufs=2) as wstpool, \
             tc.tile_pool(name="ypool", bufs=4) as ypool, \
             tc.tile_pool(name="pr", bufs=2, space="PSUM") as prpool, \
             tc.tile_pool(name="py", bufs=4, space="PSUM") as pypool, \
             tc.tile_pool(name="pw", bufs=1, space="PSUM") as pwpool:

            # ---- constants / setup ----
            eye = const.tile([P, P], f32, tag="eye")
            make_identity(nc, eye[:])

            hg = const.tile([P, H, A], f32, tag="hg")
            nc.sync.dma_start(hg[:], hg_d[:])

            idx_sb = const.tile([P, NT, A], f32, tag="idx")
            p_sb = const.tile([P, NT, A], f32, tag="p")
            # dst[tp, i, a] = src[i*P + tp, a]
            src_idx = idx_d[:].rearrange("(i tp) a -> tp i a", tp=P)
            src_p = p_d[:].rearrange("(i tp) a -> tp i a", tp=P)
            nc.sync.dma_start(idx_sb[:], src_idx)
            nc.sync.dma_start(p_sb[:], src_p)

            # routing weights wgt[tp, i, h, a] = (idx==h) * p
            wgt = const.tile([P, NT, H, A], f32, tag="wgt")
            idx_b = idx_sb[:].unsqueeze(2).broadcast_to([P, NT, H, A])
            p_b = p_sb[:].unsqueeze(2).broadcast_to([P, NT, H, A])
            hg_b = hg[:].unsqueeze(1).broadcast_to([P, NT, H, A])
            nc.vector.tensor_tensor(wgt[:], idx_b, hg_b, mybir.AluOpType.is_equal)
            nc.vector.tensor_tensor(wgt[:], wgt[:], p_b, mybir.AluOpType.mult)
            # per-head prob sums wsum[tp, i, h] = wgt[...,0] + wgt[...,1]
            wsum = const.tile([P, NT, H], f32, tag="wsum")
            nc.vector.tensor_tensor(
                wsum[:], wgt[:, :, :, 0], wgt[:, :, :, 1], mybir.AluOpType.add
            )

            # W -> SBUF, rounded to compute dtype: W_r[d, h, m].
            # Chunked (h, mc) so the first tile's matmuls can chase the
            # W stream instead of waiting for the full 8.4MB.
            w_r = const.tile([P, H, DM], cdt, tag="w_r")
            for h in range(H):
                for mc in range(MC):
                    st = wstage.tile([P, NFREE], f32, tag="wst")
                    nc.sync.dma_start(
                        st[:], w_d[h, :, mc * NFREE:(mc + 1) * NFREE]
                    )
                    dst = w_r[:, h, mc * NFREE:(mc + 1) * NFREE]
                    if mc % 2 == 0:
                        nc.vector.tensor_copy(dst, st[:])
                    else:
                        nc.scalar.copy(dst, st[:])

            # b -> SBUF rounded: b_r[h, m] (partitions 0..7)
            bstage = const.tile([H, DM], f32, tag="bstage")
            nc.sync.dma_start(bstage[:], b_d[:])
            b_r = const.tile([H, DM], cdt, tag="b_r")
            nc.vector.tensor_copy(b_r[:], bstage[:])

            # ---- main pipeline, software-pipelined by one tile ----
            prev = None  # (routedT tile, wsT tile) of tile i-1
            for i in range(NT + 1):
                cur = None
                if i < NT:
                    x_t = xpool.tile([P, A, DH], f32, tag="x")
                    nc.sync.dma_start(x_t[:], x_d[i * P:(i + 1) * P, :, :])

                    r_t = rpool.tile([P, H, DH], cdt, tag="r")
                    for g in range(2):          # head groups 0-3, 4-7
                        pr_t = prpool.tile([P, 4 * P], f32, tag="pr")
                        for hh in range(4):
                            h = g * 4 + hh
                            tmp = tmppool.tile([P, DH], f32, tag="tmp")
                            nc.vector.tensor_scalar_mul(
                                tmp[:], x_t[:, 0, :], wgt[:, i, h, 0:1]
                            )
                            nc.vector.scalar_tensor_tensor(
                                tmp[:], x_t[:, 1, :], wgt[:, i, h, 1:2], tmp[:],
                                op0=mybir.AluOpType.mult, op1=mybir.AluOpType.add,
                            )
                            nc.tensor.transpose(
                                pr_t[:, hh * P:(hh + 1) * P], tmp[:], eye[:]
                            )
                        dst = r_t[:, g * 4:(g + 1) * 4, :].rearrange("p a b -> p (a b)")
                        if g == 0:
                            nc.vector.tensor_copy(dst, pr_t[:])
                        else:
                            nc.scalar.copy(dst, pr_t[:])

                    # transposed per-head prob sums for the bias matmul
                    pw_t = pwpool.tile([H, P], f32, tag="pw")
                    nc.tensor.transpose(pw_t[:], wsum[:, i, :], eye[:])
                    ws_t = wstpool.tile([H, P], cdt, tag="ws")
                    nc.scalar.copy(ws_t[:], pw_t[:])
                    cur = (r_t, ws_t)

                if i >= 1:
                    r_p, ws_p = prev
                    j = i - 1
                    # h-outer / mc-inner: each stationary (routedT head, or
                    # the bias wsT) is loaded once and streams all 4 output
                    # chunks, amortizing LDWEIGHTS 4x.  4 PSUM banks
                    # accumulate the 4 chunks concurrently.
                    py_ts = []
                    for _mc in range(MC):
                        py_t = pypool.tile([P, NFREE], f32, tag="py")
                        py_ts.append(py_t)
                    for h in range(H):
                        for mc in range(MC):
                            nc.tensor.matmul(
                                py_ts[mc][:],
                                r_p[:, h, :],
                                w_r[:, h, mc * NFREE:(mc + 1) * NFREE],
                                start=(h == 0), stop=False,
                            )
                    for mc in range(MC):
                        nc.tensor.matmul(
                            py_ts[mc][:], ws_p[:],
                            b_r[:, mc * NFREE:(mc + 1) * NFREE],
                            start=False, stop=True,
                        )
                    for mc in range(MC):
                        y_t = ypool.tile([P, NFREE], f32, tag="y")
                        if mc % 2 == 0:
                            nc.vector.tensor_copy(y_t[:], py_ts[mc][:])
                        else:
                            nc.scalar.copy(y_t[:], py_ts[mc][:])
                        nc.sync.dma_start(
                            y_d[j * P:(j + 1) * P, mc * NFREE:(mc + 1) * NFREE],
                            y_t[:],
                        )
                prev = cur

    nc.compile()
    return nc


def _get_nc():
    if "nc" not in _CACHE:
        _CACHE["nc"] = _build_nc()
    return _CACHE["nc"]


def kernel(embedding, sel_idx, sel_probs, W, b):
    from concourse.bass_utils import run_bass_kernel_spmd

    emb = np.ascontiguousarray(embedding, dtype=np.float32).reshape(T, A, DH)
    idxf = np.ascontiguousarray(sel_idx).reshape(T, A).astype(np.float32)
    pf = np.ascontiguousarray(sel_probs, dtype=np.float32).reshape(T, A)
    Wf = np.ascontiguousarray(W, dtype=np.float32)
    bf = np.ascontiguousarray(b, dtype=np.float32)
    hgrid = np.ascontiguousarray(
        np.broadcast_to(
            np.arange(H, dtype=np.float32)[None, :, None], (P, H, A)
        )
    )

    nc = _get_nc()
    in_maps = []
    for c in range(NCORES):
        sl = slice(c * TLOC, (c + 1) * TLOC)
        in_maps.append({
            "x": emb[sl],
            "idxf": idxf[sl],
            "p": pf[sl],
            "W": Wf,
            "b": bf,
            "hgrid": hgrid,
        })

    trace = os.environ.get("TRNK_TRACE") == "1"
    if trace:
        _register_ntff_stub()
    res = run_bass_kernel_spmd(
        nc, in_maps, core_ids=list(range(NCORES)), trace=trace
    )
    if trace:
        _CACHE["exec_time_ns"] = res.exec_time_ns
        _CACHE["results_obj"] = res

    out = np.concatenate(
        [res.results[c]["out"] for c in range(NCORES)], axis=0
    )
    return out.reshape(B, S, DM)


def _register_ntff_stub():
    """antenv.axon_hooks is absent in this image; back it with the boot
    ctypes NTFF hook so trace=True works under axon."""
    import sys, types
    try:
        import antenv.axon_hooks  # noqa: F401
        return
    except ImportError:
        pass
    try:
        import antenv
        from trn_agent_boot.trn_boot import _ntff_profile_via_ctypes
    except ImportError:
        return
    mod = types.ModuleType("antenv.axon_hooks")
    hook = [None]

    def set_axon_ntff_profile_hook(h):
        hook[0] = h

    def get_axon_ntff_profile_hook():
        if hook[0] is None:
            hook[0] = _ntff_profile_via_ctypes("/opt/axon/libaxon_pjrt.so")
        return hook[0]

    mod.set_axon_ntff_profile_hook = set_axon_ntff_profile_hook
    mod.get_axon_ntff_profile_hook = get_axon_ntff_profile_hook
    sys.modules["antenv.axon_hooks"] = mod
    antenv.axon_hooks = mod


# revision 8
# speedup vs baseline: 1.1568x; 1.0391x over previous
"""Trainium2 Bass kernel for nn_MergeHeads (moe_routing).

Computes out[t] = sum_a p[t,a] * (x[t,a] @ W[idx[t,a]] + b[idx[t,a]])
for B*S = 16384 tokens, A=2 slots, H=8 heads, DH=128, DM=2048.

Strategy: data-parallel over tokens across 8 NeuronCores (2048 tokens
each); W/b replicated.  Per core, dense routed formulation:
  routedT[h][d, t] = sum_a (idx[t,a]==h) * p[t,a] * x[t,a,d]
built with DVE mask-multiplies (token-on-partition layout, per-partition
scalars) + one PE transpose per (tile, head) accumulating into PSUM;
the mandatory PSUM->SBUF copy rounds to float32r.  Then per 512-wide
output chunk: 8 accumulated float32r matmuls (K=128, tokens stationary,
W moving N=512) plus one K=8 bias matmul (lhsT = transposed per-head
prob sums, rhs = b).  float32r runs at full PE rate (1 cyc/row at
N>=256) with ~1.6e-4 max relative error vs fp32.
"""

import os
import numpy as np

B, S, A, H, DH, DM = 4, 4096, 2, 8, 128, 2048
NCORES = 8
T = B * S
TLOC = T // NCORES        # 2048 tokens per core
P = 128                   # partitions / token tile
NT = TLOC // P            # 16 token tiles per core
NFREE = 512               # matmul moving free dim (one PSUM bank of fp32)
MC = DM // NFREE          # 4 output chunks per token tile

# compute dtype: "f32r" (default), "bf16", or "f32"
CDT_MODE = os.environ.get("TRNK_DTYPE", "f32r")

_CACHE = {}


def _build_nc():
    import concourse.mybir as mybir
    from concourse import bacc
    from concourse.tile import TileContext
    from concourse.masks import make_identity

    f32 = mybir.dt.float32
    cdt = {
        "f32r": mybir.dt.float32r,
        "bf16": mybir.dt.bfloat16,
        "f32": mybir.dt.float32,
    }[CDT_MODE]

    nc = bacc.Bacc("TRN2", target_bir_lowering=False, debug=False)

    x_d = nc.dram_tensor("x", [TLOC, A, DH], f32, kind="ExternalInput")
    idx_d = nc.dram_tensor("idxf", [TLOC, A], f32, kind="ExternalInput")
    p_d = nc.dram_tensor("p", [TLOC, A], f32, kind="ExternalInput")
    w_d = nc.dram_tensor("W", [H, DH, DM], f32, kind="ExternalInput")
    b_d = nc.dram_tensor("b", [H, DM], f32, kind="ExternalInput")
    hg_d = nc.dram_tensor("hgrid", [P, H, A], f32, kind="ExternalInput")
    y_d = nc.dram_tensor("out", [TLOC, DM], f32, kind="ExternalOutput")

    with TileContext(nc) as tc:
        with tc.tile_pool(name="const", bufs=1) as const, \
             tc.tile_pool(name="wstage", bufs=2) as wstage, \
             tc.tile_pool(name="xpool", bufs=4) as xpool, \
             tc.tile_pool(name="tmp", bufs=6) as tmppool, \
             tc.tile_pool(name="rpool", bufs=2) as rpool, \
             tc.tile_pool(name="wst", bufs=2) as wstpool, \
             tc.tile_pool(name="ypool", bufs=4) as ypool, \
             tc.tile_pool(name="pr", bufs=2, space="PSUM") as prpool, \
             tc.tile_pool(name="py", bufs=4, space="PSUM") as pypool, \
             tc.tile_pool(name="pw", bufs=1, space="PSUM") as pwpool:

            # ---- constants / setup ----
            eye = const.tile([P, P], f32, tag="eye")
            make_identity(nc, eye[:])

            hg = const.tile([P, H, A], f32, tag="hg")
            nc.sync.dma_start(hg[:], hg_d[:])

            idx_sb = const.tile([P, NT, A], f32, tag="idx")
            p_sb = const.tile([P, NT, A], f32, tag="p")
            # dst[tp, i, a] = src[i*P + tp, a]
            src_idx = idx_d[:].rearrange("(i tp) a -> tp i a", tp=P)
            src_p = p_d[:].rearrange("(i tp) a -> tp i a", tp=P)
            nc.sync.dma_start(idx_sb[:], src_idx)
            nc.sync.dma_start(p_sb[:], src_p)

            # routing weights wgt[tp, i, h, a] = (idx==h) * p
            wgt = const.tile([P, NT, H, A], f32, tag="wgt")
            idx_b = idx_sb[:].unsqueeze(2).broadcast_to([P, NT, H, A])
            p_b = p_sb[:].unsqueeze(2).broadcast_to([P, NT, H, A])
            hg_b = hg[:].unsqueeze(1).broadcast_to([P, NT, H, A])
            nc.vector.tensor_tensor(wgt[:], idx_b, hg_b, mybir.AluOpType.is_equal)
            nc.vector.tensor_tensor(wgt[:], wgt[:], p_b, mybir.AluOpType.mult)
            # per-head prob sums wsum[tp, i, h] = wgt[...,0] + wgt[...,1]
            wsum = const.tile([P, NT, H], f32, tag="wsum")
            nc.vector.tensor_tensor(
                wsum[:], wgt[:, :, :, 0], wgt[:, :, :, 1], mybir.AluOpType.add
            )

            # W -> SBUF, rounded to compute dtype: W_r[d, h, m].
            # Chunked (h, mc) so the first tile's matmuls can chase the
            # W stream instead of waiting for the full 8.4MB.
            w_r = const.tile([P, H, DM], cdt, tag="w_r")
            for h in range(H):
                for mc in range(MC):
                    st = wstage.tile([P, NFREE], f32, tag="wst")
                    nc.sync.dma_start(
                        st[:], w_d[h, :, mc * NFREE:(mc + 1) * NFREE]
                    )
                    dst = w_r[:, h, mc * NFREE:(mc + 1) * NFREE]
                    if mc % 2 == 0:
                        nc.vector.tensor_copy(dst, st[:])
                    else:
                        nc.scalar.copy(dst, st[:])

            # b -> SBUF rounded: b_r[h, m] (partitions 0..7)
            bstage = const.tile([H, DM], f32, tag="bstage")
            nc.sync.dma_start(bstage[:], b_d[:])
            b_r = const.tile([H, DM], cdt, tag="b_r")
            nc.vector.tensor_copy(b_r[:], bstage[:])

            # ---- main pipeline, software-pipelined by one tile ----
            prev = None  # (routedT tile, wsT tile) of tile i-1
            for i in range(NT + 1):
                cur = None
                if i < NT:
                    x_t = xpool.tile([P, A, DH], f32, tag="x")
                    nc.sync.dma_start(x_t[:], x_d[i * P:(i + 1) * P, :, :])

                    r_t = rpool.tile([P, H, DH], cdt, tag="r")
                    for g in range(2):          # head groups 0-3, 4-7
                        pr_t = prpool.tile([P, 4 * P], f32, tag="pr")
                        for hh in range(4):
                            h = g * 4 + hh
                            tmp = tmppool.tile([P, DH], f32, tag="tmp")
                            nc.vector.tensor_scalar_mul(
                                tmp[:], x_t[:, 0, :], wgt[:, i, h, 0:1]
                            )
                            nc.vector.scalar_tensor_tensor(
                                tmp[:], x_t[:, 1, :], wgt[:, i, h, 1:2], tmp[:],
                                op0=mybir.AluOpType.mult, op1=mybir.AluOpType.add,
                            )
                            nc.tensor.transpose(
                                pr_t[:, hh * P:(hh + 1) * P], tmp[:], eye[:]
                            )
                        dst = r_t[:, g * 4:(g + 1) * 4, :].rearrange("p a b -> p (a b)")
                        if g == 0:
                            nc.vector.tensor_copy(dst, pr_t[:])
                        else:
                            nc.scalar.copy(dst, pr_t[:])

                    # transposed per-head prob sums for the bias matmul
                    pw_t = pwpool.tile([H, P], f32, tag="pw")
                    nc.tensor.transpose(pw_t[:], wsum[:, i, :], eye[:])
                    ws_t = wstpool.tile([H, P], cdt, tag="ws")
                    nc.scalar.copy(ws_t[:], pw_t[:])
                    cur = (r_t, ws_t)

                if i >= 1:
                    r_p, ws_p = prev
                    j = i - 1
                    # h-outer / mc-inner: each stationary (routedT head, or
                    # the bias wsT) is loaded once and streams all 4 output
                    # chunks, amortizing LDWEIGHTS 4x.  4 PSUM banks
                    # accumulate the 4 chunks concurrently.
                    py_ts = []
                    for _mc in range(MC):
                        py_t = pypool.tile([P, NFREE], f32, tag="py")
                        py_ts.append(py_t)
                    for h in range(H):
                        for mc in range(MC):
                            nc.tensor.matmul(
                                py_ts[mc][:],
                                r_p[:, h, :],
                                w_r[:, h, mc * NFREE:(mc + 1) * NFREE],
                                start=(h == 0), stop=False,
                            )
                    for mc in range(MC):
                        nc.tensor.matmul(
                            py_ts[mc][:], ws_p[:],
                            b_r[:, mc * NFREE:(mc + 1) * NFREE],
                            start=False, stop=True,
                        )
                    for mc in range(MC):
                        y_t = ypool.tile([P, NFREE], f32, tag="y")
                        if mc % 2 == 0:
                            nc.vector.tensor_copy(y_t[:], py_ts[mc][:])
                        else:
                            nc.scalar.copy(y_t[:], py_ts[mc][:])
                        nc.gpsimd.dma_start(
                            y_d[j * P:(j + 1) * P, mc * NFREE:(mc + 1) * NFREE],
                            y_t[:],
                        )
                prev = cur

    nc.compile()
    return nc


def _get_nc():
    if "nc" not in _CACHE:
        _CACHE["nc"] = _build_nc()
    return _CACHE["nc"]


def kernel(embedding, sel_idx, sel_probs, W, b):
    from concourse.bass_utils import run_bass_kernel_spmd

    emb = np.ascontiguousarray(embedding, dtype=np.float32).reshape(T, A, DH)
    idxf = np.ascontiguousarray(sel_idx).reshape(T, A).astype(np.float32)
    pf = np.ascontiguousarray(sel_probs, dtype=np.float32).reshape(T, A)
    Wf = np.ascontiguousarray(W, dtype=np.float32)
    bf = np.ascontiguousarray(b, dtype=np.float32)
    hgrid = np.ascontiguousarray(
        np.broadcast_to(
            np.arange(H, dtype=np.float32)[None, :, None], (P, H, A)
        )
    )

    nc = _get_nc()
    in_maps = []
    for c in range(NCORES):
        sl = slice(c * TLOC, (c + 1) * TLOC)
        in_maps.append({
            "x": emb[sl],
            "idxf": idxf[sl],
            "p": pf[sl],
            "W": Wf,
            "b": bf,
            "hgrid": hgrid,
        })

    trace = os.environ.get("TRNK_TRACE") == "1"
    if trace:
        _register_ntff_stub()
    res = run_bass_kernel_spmd(
        nc, in_maps, core_ids=list(range(NCORES)), trace=trace
    )
    if trace:
        _CACHE["exec_time_ns"] = res.exec_time_ns
        _CACHE["results_obj"] = res

    out = np.concatenate(
        [res.results[c]["out"] for c in range(NCORES)], axis=0
    )
    return out.reshape(B, S, DM)


def _register_ntff_stub():
    """antenv.axon_hooks is absent in this image; back it with the boot
    ctypes NTFF hook so trace=True works under axon."""
    import sys, types
    try:
        import antenv.axon_hooks  # noqa: F401
        return
    except ImportError:
        pass
    try:
        import antenv
        from trn_agent_boot.trn_boot import _ntff_profile_via_ctypes
    except ImportError:
        return
    mod = types.ModuleType("antenv.axon_hooks")
    hook = [None]

    def set_axon_ntff_profile_hook(h):
        hook[0] = h

    def get_axon_ntff_profile_hook():
        if hook[0] is None:
            hook[0] = _ntff_profile_via_ctypes("/opt/axon/libaxon_pjrt.so")
        return hook[0]

    mod.set_axon_ntff_profile_hook = set_axon_ntff_profile_hook
    mod.get_axon_ntff_profile_hook = get_axon_ntff_profile_hook
    sys.modules["antenv.axon_hooks"] = mod
    antenv.axon_hooks = mod


# revision 9
# speedup vs baseline: 1.3592x; 1.1750x over previous
"""Trainium2 Bass kernel for nn_MergeHeads (moe_routing).

Computes out[t] = sum_a p[t,a] * (x[t,a] @ W[idx[t,a]] + b[idx[t,a]])
for B*S = 16384 tokens, A=2 slots, H=8 heads, DH=128, DM=2048.

Strategy: data-parallel over tokens across 8 NeuronCores (2048 tokens
each); W/b replicated.  Per core, dense routed formulation:
  routedT[h][d, t] = sum_a (idx[t,a]==h) * p[t,a] * x[t,a,d]
built with DVE mask-multiplies (token-on-partition layout, per-partition
scalars) + one PE transpose per (tile, head) accumulating into PSUM;
the mandatory PSUM->SBUF copy rounds to float32r.  Then per 512-wide
output chunk: 8 accumulated float32r matmuls (K=128, tokens stationary,
W moving N=512) plus one K=8 bias matmul (lhsT = transposed per-head
prob sums, rhs = b).  float32r runs at full PE rate (1 cyc/row at
N>=256) with ~1.6e-4 max relative error vs fp32.
"""

import os
import numpy as np

B, S, A, H, DH, DM = 4, 4096, 2, 8, 128, 2048
NCORES = 8
T = B * S
TLOC = T // NCORES        # 2048 tokens per core
P = 128                   # partitions / token tile
NT = TLOC // P            # 16 token tiles per core
NFREE = 512               # matmul moving free dim (one PSUM bank of fp32)
MC = DM // NFREE          # 4 output chunks per token tile

# compute dtype: "f32r" (default), "bf16", or "f32"
CDT_MODE = os.environ.get("TRNK_DTYPE", "f32r")

_CACHE = {}


def _build_nc():
    import concourse.mybir as mybir
    from concourse import bacc
    from concourse.tile import TileContext
    from concourse.masks import make_identity

    f32 = mybir.dt.float32
    cdt = {
        "f32r": mybir.dt.float32r,
        "bf16": mybir.dt.bfloat16,
        "f32": mybir.dt.float32,
    }[CDT_MODE]

    nc = bacc.Bacc("TRN2", target_bir_lowering=False, debug=False)

    x_d = nc.dram_tensor("x", [TLOC, A, DH], f32, kind="ExternalInput")
    idx_d = nc.dram_tensor("idxf", [TLOC, A], f32, kind="ExternalInput")
    p_d = nc.dram_tensor("p", [TLOC, A], f32, kind="ExternalInput")
    w_d = nc.dram_tensor("W", [H, DH, DM], f32, kind="ExternalInput")
    b_d = nc.dram_tensor("b", [H, DM], f32, kind="ExternalInput")
    hg_d = nc.dram_tensor("hgrid", [P, H, A], f32, kind="ExternalInput")
    y_d = nc.dram_tensor("out", [TLOC, DM], f32, kind="ExternalOutput")

    with TileContext(nc) as tc:
        with tc.tile_pool(name="const", bufs=1) as const, \
             tc.tile_pool(name="wstage", bufs=2) as wstage, \
             tc.tile_pool(name="xpool", bufs=4) as xpool, \
             tc.tile_pool(name="tmp", bufs=6) as tmppool, \
             tc.tile_pool(name="rpool", bufs=2) as rpool, \
             tc.tile_pool(name="wst", bufs=2) as wstpool, \
             tc.tile_pool(name="ypool", bufs=3) as ypool, \
             tc.tile_pool(name="pr", bufs=2, space="PSUM") as prpool, \
             tc.tile_pool(name="py", bufs=4, space="PSUM") as pypool, \
             tc.tile_pool(name="pw", bufs=1, space="PSUM") as pwpool:

            # ---- constants / setup ----
            eye = const.tile([P, P], f32, tag="eye")
            make_identity(nc, eye[:])

            hg = const.tile([P, H, A], f32, tag="hg")
            nc.sync.dma_start(hg[:], hg_d[:])

            idx_sb = const.tile([P, NT, A], f32, tag="idx")
            p_sb = const.tile([P, NT, A], f32, tag="p")
            # dst[tp, i, a] = src[i*P + tp, a]
            src_idx = idx_d[:].rearrange("(i tp) a -> tp i a", tp=P)
            src_p = p_d[:].rearrange("(i tp) a -> tp i a", tp=P)
            nc.sync.dma_start(idx_sb[:], src_idx)
            nc.sync.dma_start(p_sb[:], src_p)

            # routing weights wgt[tp, i, h, a] = (idx==h) * p
            wgt = const.tile([P, NT, H, A], f32, tag="wgt")
            idx_b = idx_sb[:].unsqueeze(2).broadcast_to([P, NT, H, A])
            p_b = p_sb[:].unsqueeze(2).broadcast_to([P, NT, H, A])
            hg_b = hg[:].unsqueeze(1).broadcast_to([P, NT, H, A])
            nc.vector.tensor_tensor(wgt[:], idx_b, hg_b, mybir.AluOpType.is_equal)
            nc.vector.tensor_tensor(wgt[:], wgt[:], p_b, mybir.AluOpType.mult)
            # per-head prob sums wsum[tp, i, h] = wgt[...,0] + wgt[...,1]
            wsum = const.tile([P, NT, H], f32, tag="wsum")
            nc.vector.tensor_tensor(
                wsum[:], wgt[:, :, :, 0], wgt[:, :, :, 1], mybir.AluOpType.add
            )

            # W -> SBUF, rounded to compute dtype: W_r[d, h, m].
            # Chunked (h, mc) so the first tile's matmuls can chase the
            # W stream instead of waiting for the full 8.4MB.
            w_r = const.tile([P, H, DM], cdt, tag="w_r")
            for h in range(H):
                st = wstage.tile([P, DM], f32, tag="wst")
                nc.sync.dma_start(st[:], w_d[h, :, :])
                if h % 2 == 0:
                    nc.vector.tensor_copy(w_r[:, h, :], st[:])
                else:
                    nc.scalar.copy(w_r[:, h, :], st[:])

            # b -> SBUF rounded: b_r[h, m] (partitions 0..7)
            bstage = const.tile([H, DM], f32, tag="bstage")
            nc.sync.dma_start(bstage[:], b_d[:])
            b_r = const.tile([H, DM], cdt, tag="b_r")
            nc.vector.tensor_copy(b_r[:], bstage[:])

            # ---- main pipeline, software-pipelined by one tile ----
            prev = None  # (routedT tile, wsT tile) of tile i-1
            for i in range(NT + 1):
                cur = None
                if i < NT:
                    x_t = xpool.tile([P, A, DH], f32, tag="x")
                    nc.sync.dma_start(x_t[:], x_d[i * P:(i + 1) * P, :, :])

                    r_t = rpool.tile([P, H, DH], cdt, tag="r")
                    for g in range(2):          # head groups 0-3, 4-7
                        pr_t = prpool.tile([P, 4 * P], f32, tag="pr")
                        for hh in range(4):
                            h = g * 4 + hh
                            tmp = tmppool.tile([P, DH], f32, tag="tmp")
                            nc.vector.tensor_scalar_mul(
                                tmp[:], x_t[:, 0, :], wgt[:, i, h, 0:1]
                            )
                            nc.vector.scalar_tensor_tensor(
                                tmp[:], x_t[:, 1, :], wgt[:, i, h, 1:2], tmp[:],
                                op0=mybir.AluOpType.mult, op1=mybir.AluOpType.add,
                            )
                            nc.tensor.transpose(
                                pr_t[:, hh * P:(hh + 1) * P], tmp[:], eye[:]
                            )
                        dst = r_t[:, g * 4:(g + 1) * 4, :].rearrange("p a b -> p (a b)")
                        if g == 0:
                            nc.vector.tensor_copy(dst, pr_t[:])
                        else:
                            nc.scalar.copy(dst, pr_t[:])

                    # transposed per-head prob sums for the bias matmul
                    pw_t = pwpool.tile([H, P], f32, tag="pw")
                    nc.tensor.transpose(pw_t[:], wsum[:, i, :], eye[:])
                    ws_t = wstpool.tile([H, P], cdt, tag="ws")
                    nc.scalar.copy(ws_t[:], pw_t[:])
                    cur = (r_t, ws_t)

                if i >= 1:
                    r_p, ws_p = prev
                    j = i - 1
                    # h-outer / mc-inner: each stationary (routedT head, or
                    # the bias wsT) is loaded once and streams all 4 output
                    # chunks, amortizing LDWEIGHTS 4x.  4 PSUM banks
                    # accumulate the 4 chunks concurrently.
                    py_ts = []
                    for _mc in range(MC):
                        py_t = pypool.tile([P, NFREE], f32, tag="py")
                        py_ts.append(py_t)
                    for h in range(H):
                        for mc in range(MC):
                            nc.tensor.matmul(
                                py_ts[mc][:],
                                r_p[:, h, :],
                                w_r[:, h, mc * NFREE:(mc + 1) * NFREE],
                                start=(h == 0), stop=False,
                            )
                    for mc in range(MC):
                        nc.tensor.matmul(
                            py_ts[mc][:], ws_p[:],
                            b_r[:, mc * NFREE:(mc + 1) * NFREE],
                            start=False, stop=True,
                        )
                    y_t = ypool.tile([P, DM], f32, tag="y")
                    for mc in range(MC):
                        dst = y_t[:, mc * NFREE:(mc + 1) * NFREE]
                        if mc % 2 == 0:
                            nc.vector.tensor_copy(dst, py_ts[mc][:])
                        else:
                            nc.scalar.copy(dst, py_ts[mc][:])
                    nc.gpsimd.dma_start(y_d[j * P:(j + 1) * P, :], y_t[:])
                prev = cur

    nc.compile()
    return nc


def _get_nc():
    if "nc" not in _CACHE:
        _CACHE["nc"] = _build_nc()
    return _CACHE["nc"]


def kernel(embedding, sel_idx, sel_probs, W, b):
    from concourse.bass_utils import run_bass_kernel_spmd

    emb = np.ascontiguousarray(embedding, dtype=np.float32).reshape(T, A, DH)
    idxf = np.ascontiguousarray(sel_idx).reshape(T, A).astype(np.float32)
    pf = np.ascontiguousarray(sel_probs, dtype=np.float32).reshape(T, A)
    Wf = np.ascontiguousarray(W, dtype=np.float32)
    bf = np.ascontiguousarray(b, dtype=np.float32)
    hgrid = np.ascontiguousarray(
        np.broadcast_to(
            np.arange(H, dtype=np.float32)[None, :, None], (P, H, A)
        )
    )

    nc = _get_nc()
    in_maps = []
    for c in range(NCORES):
        sl = slice(c * TLOC, (c + 1) * TLOC)
        in_maps.append({
            "x": emb[sl],
            "idxf": idxf[sl],
            "p": pf[sl],
            "W": Wf,
            "b": bf,
            "hgrid": hgrid,
        })

    trace = os.environ.get("TRNK_TRACE") == "1"
    if trace:
        _register_ntff_stub()
    res = run_bass_kernel_spmd(
        nc, in_maps, core_ids=list(range(NCORES)), trace=trace
    )
    if trace:
        _CACHE["exec_time_ns"] = res.exec_time_ns
        _CACHE["results_obj"] = res

    out = np.concatenate(
        [res.results[c]["out"] for c in range(NCORES)], axis=0
    )
    return out.reshape(B, S, DM)


def _register_ntff_stub():
    """antenv.axon_hooks is absent in this image; back it with the boot
    ctypes NTFF hook so trace=True works under axon."""
    import sys, types
    try:
        import antenv.axon_hooks  # noqa: F401
        return
    except ImportError:
        pass
    try:
        import antenv
        from trn_agent_boot.trn_boot import _ntff_profile_via_ctypes
    except ImportError:
        return
    mod = types.ModuleType("antenv.axon_hooks")
    hook = [None]

    def set_axon_ntff_profile_hook(h):
        hook[0] = h

    def get_axon_ntff_profile_hook():
        if hook[0] is None:
            hook[0] = _ntff_profile_via_ctypes("/opt/axon/libaxon_pjrt.so")
        return hook[0]

    mod.set_axon_ntff_profile_hook = set_axon_ntff_profile_hook
    mod.get_axon_ntff_profile_hook = get_axon_ntff_profile_hook
    sys.modules["antenv.axon_hooks"] = mod
    antenv.axon_hooks = mod
